# revision 6
# baseline (speedup 1.0000x reference)
"""AdaptiveTemporalVQ eval-forward on 8 TRN2 NeuronCores (Bass/Tile).

Data-parallel over batch B=16: each core gets 2 batches (16384 tokens).
The codebook (8192x512), boundary Linear, and derived constants are
replicated to every core. Scalar losses are combined on the host from
per-core partial sums.

Per-core device pipeline:
  stage A  : x tiles [128 tok, 512] -> PE pooling matmuls (x as lhsT,
             P8 selector as rhs) -> pooledT; PE transposes -> pooled_nat;
             boundary logits via fused affine_mul_reduce on DVE.
  stage B  : scores s[seg, code] = 2*pooled.e - |e|^2 in two code halves
             (codebook half resident in SBUF at a time); fp32 PE matmuls
             accumulate 4 d-chunks in PSUM; DVE evicts with the esq
             subtract fused; vector.max + max_index give the argmax
             (= VQ argmin) per segment; halves combined with
             first-occurrence tie semantics.
  stage C  : indices -> int16 wrap layout -> one dma_gather pulls the
             2048 selected codebook rows; delta/q_st/partial-sq on DVE;
             quantized_out expanded over SPAN=8 by strided DMA writes.

Outputs per core: qout [16384,512] f32, idx [2048,8] i32 (token-expanded),
bnd [128,128] f32 (token-transposed), sq [128,16] f32 partial sums.
"""
import numpy as np

NUM_EMB = 8192
EMB_DIM = 512
SPAN = 8
COMMIT = 0.25
BOUND_W = 0.01
B, T = 16, 8192
NCORES = 8
B_SH = B // NCORES              # 2 batches per core
TOK = B_SH * T                  # 16384 tokens per core
SEG = TOK // SPAN               # 2048 segments per core
P = 128
NTT = TOK // P                  # 128 token tiles
NST = SEG // P                  # 16 segment tiles
TT_PER_ST = P // (P // SPAN)    # 8 token tiles per segment tile
SEGS_PER_TT = P // SPAN         # 16 segments per token tile
DCH = EMB_DIM // P              # 4 contraction chunks
NHALF = 2
KHALF = NUM_EMB // NHALF        # 4096 codes per half
NCH = KHALF // 512              # 8 psum chunks of 512 codes per half

_CACHE = {}


def _build():
    import os
    STAGE = os.environ.get("VQ_STAGE", "FULL")
    import concourse.bass as bass
    import concourse.tile as tile
    from concourse import bacc, mybir
    from concourse.masks import make_identity
    from concourse.dve_ops import TENSOR_TENSOR_REDUCE

    f32 = mybir.dt.float32
    i32 = mybir.dt.int32
    i16 = mybir.dt.int16
    u32 = mybir.dt.uint32
    Alu = mybir.AluOpType

    nc = bacc.Bacc("TRN2", target_bir_lowering=False, debug=False)

    d_x = nc.dram_tensor("x", [TOK, EMB_DIM], f32, kind="ExternalInput")
    d_e2T = nc.dram_tensor("e2T", [EMB_DIM, NUM_EMB], f32, kind="ExternalInput")
    d_emb = nc.dram_tensor("emb", [NUM_EMB, EMB_DIM], f32, kind="ExternalInput")
    d_esq = nc.dram_tensor("esq", [P, NUM_EMB], f32, kind="ExternalInput")
    d_wb = nc.dram_tensor("wb", [P, EMB_DIM], f32, kind="ExternalInput")
    d_bb = nc.dram_tensor("bb", [P, 1], f32, kind="ExternalInput")
    d_p8 = nc.dram_tensor("p8", [P, SEGS_PER_TT], f32, kind="ExternalInput")

    d_qout = nc.dram_tensor("qout", [TOK, EMB_DIM], f32, kind="ExternalOutput")
    d_idx = nc.dram_tensor("idx", [SEG, SPAN], i32, kind="ExternalOutput")
    d_bnd = nc.dram_tensor("bnd", [NTT, P], f32, kind="ExternalOutput")
    d_sq = nc.dram_tensor("sq", [P, NST], f32, kind="ExternalOutput")

    d_i16s = nc.dram_tensor("i16s", [SEG], i16)   # internal scratch

    with tile.TileContext(nc) as tc:
        with tc.tile_pool(name="long", bufs=1) as lp, \
             tc.tile_pool(name="ps_pool", bufs=2, space="PSUM") as pp, \
             tc.tile_pool(name="ps_tp", bufs=2, space="PSUM") as tp, \
             tc.tile_pool(name="ps_sc", bufs=4, space="PSUM") as scp:

            # ---- constants
            p8_sb = lp.tile([P, SEGS_PER_TT], f32, tag="p8")
            nc.gpsimd.dma_start(p8_sb[:], d_p8[:])
            wb_sb = lp.tile([P, EMB_DIM], f32, tag="wb")
            nc.gpsimd.dma_start(wb_sb[:], d_wb[:])
            bb_sb = lp.tile([P, 1], f32, tag="bb")
            nc.gpsimd.dma_start(bb_sb[:], d_bb[:])
            ident = lp.tile([P, P], f32, tag="ident")
            make_identity(nc, ident[:])

            # ---- long-lived accumulators / results
            pooledT = lp.tile([P, NST, DCH, P], f32, tag="pooledT")
            pooled_nat = lp.tile([P, NST, EMB_DIM], f32, tag="pooled_nat")
            logit_all = lp.tile([P, NTT], f32, tag="logit")
            gm_acc = lp.tile([P, NHALF, NST], f32, tag="gm")
            idxf_acc = lp.tile([P, NHALF, NST], f32, tag="idxf")
            sq_acc = lp.tile([P, NST], f32, tag="sq")
            nc.vector.memset(sq_acc[:], 0.0)

            # ================= stage A: pooling + transposes + logits
            with tc.tile_pool(name="xa", bufs=3) as xap:
                for tt in range(NTT):
                    x_t = xap.tile([P, EMB_DIM], f32, tag="x")
                    nc.sync.dma_start(x_t[:], d_x[tt * P:(tt + 1) * P, :])
                    st, ttl = tt // TT_PER_ST, tt % TT_PER_ST
                    ps_pool = pp.tile([P, DCH, SEGS_PER_TT], f32, tag="pool")
                    for c in range(DCH):
                        nc.tensor.matmul(ps_pool[:, c, :],
                                         lhsT=x_t[:, c * P:(c + 1) * P],
                                         rhs=p8_sb[:], start=True, stop=True)
                    nc.scalar.mul(pooledT[:, st, :, ttl * SEGS_PER_TT:(ttl + 1) * SEGS_PER_TT],
                                  ps_pool[:], 1.0)
                    scr = xap.tile([P, EMB_DIM], f32, tag="scr")
                    nc.vector.affine_mul_reduce(
                        out=scr[:], accum_out=logit_all[:, tt:tt + 1],
                        in0=x_t[:], in1=wb_sb[:], scale=1.0, bias=0.0)
                # pooled_nat via PE transposes of pooledT
                for st in range(NST):
                    for c in range(DCH):
                        ps_t = tp.tile([P, P], f32, tag="tp")
                        nc.tensor.transpose(out=ps_t[:],
                                            in_=pooledT[:, st, c, :],
                                            identity=ident[:])
                        nc.scalar.mul(pooled_nat[:, st, c * P:(c + 1) * P],
                                      ps_t[:], 1.0)

            # ================= stage B: scores + argmax per half
            for h in range(NHALF):
                with tc.tile_pool(name=f"half{h}", bufs=1) as hp:
                    e2T_sb = hp.tile([P, DCH, KHALF], f32, tag="e2T")
                    nc.gpsimd.dma_start(
                        e2T_sb[:],
                        d_e2T[:].rearrange("(c p) k -> p c k", c=DCH)
                        [:, :, h * KHALF:(h + 1) * KHALF])
                    esq_sb = hp.tile([P, KHALF], f32, tag="esq")
                    nc.gpsimd.dma_start(esq_sb[:],
                                        d_esq[:, h * KHALF:(h + 1) * KHALF])
                    for st in range(NST):
                        score = hp.tile([P, KHALF], f32, tag="score")
                        for ch in range(NCH):
                            ps_s = scp.tile([P, 512], f32, tag="sc")
                            for c in range(DCH):
                                nc.tensor.matmul(
                                    ps_s[:],
                                    lhsT=pooledT[:, st, c, :],
                                    rhs=e2T_sb[:, c, ch * 512:(ch + 1) * 512],
                                    start=(c == 0), stop=(c == DCH - 1))
                            nc.vector.tensor_tensor(
                                out=score[:, ch * 512:(ch + 1) * 512],
                                in0=ps_s[:],
                                in1=esq_sb[:, ch * 512:(ch + 1) * 512],
                                op=Alu.subtract)
                        m8 = hp.tile([P, 8], f32, tag="m8")
                        nc.vector.max(out=m8[:], in_=score[:])
                        i8 = hp.tile([P, 8], u32, tag="i8")
                        nc.vector.max_index(out=i8[:], in_max=m8[:], in_values=score[:])
                        nc.vector.tensor_copy(out=gm_acc[:, h, st:st + 1],
                                              in_=m8[:, 0:1])
                        nc.vector.tensor_scalar(
                            out=idxf_acc[:, h, st:st + 1], in0=i8[:, 0:1],
                            scalar1=float(h * KHALF), scalar2=None, op0=Alu.add)

            # ================= combine halves -> final idx per segment
            with tc.tile_pool(name="fin", bufs=2) as fp:
                idx_i32 = lp.tile([P, NST], i32, tag="idx32")
                idx_i16 = lp.tile([P, NST], i16, tag="idx16")
                for st in range(NST):
                    gm = fp.tile([P, 1], f32, tag="cgm")
                    nc.vector.tensor_reduce(out=gm[:], in_=gm_acc[:, :, st],
                                            axis=mybir.AxisListType.X, op=Alu.max)
                    mask = fp.tile([P, NHALF], f32, tag="cmask")
                    nc.vector.tensor_tensor(out=mask[:], in0=gm_acc[:, :, st],
                                            in1=gm[:, 0:1].to_broadcast([P, NHALF]),
                                            op=Alu.is_ge)
                    pen = fp.tile([P, NHALF], f32, tag="cpen")
                    nc.vector.tensor_scalar(out=pen[:], in0=mask[:],
                                            scalar1=-1.0e9, scalar2=1.0e9,
                                            op0=Alu.mult, op1=Alu.add)
                    cand = fp.tile([P, NHALF], f32, tag="ccand")
                    nc.vector.tensor_tensor(out=cand[:], in0=idxf_acc[:, :, st],
                                            in1=pen[:], op=Alu.add)
                    idxf = fp.tile([P, 1], f32, tag="cidxf")
                    nc.vector.tensor_reduce(out=idxf[:], in_=cand[:],
                                            axis=mybir.AxisListType.X, op=Alu.min)
                    nc.vector.tensor_copy(out=idx_i32[:, st:st + 1], in_=idxf[:])
                    nc.vector.tensor_copy(out=idx_i16[:, st:st + 1], in_=idxf[:])
                    i8rep = fp.tile([P, SPAN], i32, tag="i8rep")
                    nc.vector.tensor_copy(
                        out=i8rep[:],
                        in_=idx_i32[:, st:st + 1].to_broadcast([P, SPAN]))
                    nc.sync.dma_start(d_idx[st * P:(st + 1) * P, :], i8rep[:])

                # idx -> DRAM in seg order, then back as the 16-wrapped,
                # 8x-replicated int16 layout dma_gather wants
                if STAGE != "AB":
                  nc.gpsimd.dma_start(
                    d_i16s[:].rearrange("(s p) -> p s", p=P), idx_i16[:])
                  idxs16 = lp.tile([P, SEG // 16], i16, tag="idxs16")
                  for g8 in range(8):
                    nc.gpsimd.dma_start(
                        idxs16[16 * g8:16 * (g8 + 1), :],
                        d_i16s[:].rearrange("(s p) -> p s", p=16))

                # ============= stage C: gather + outputs
                if STAGE != "AB":
                  q_all = lp.tile([P, NST, EMB_DIM], f32, tag="qall")
                  if STAGE == "ABW":
                    for _st in range(NST):
                        nc.gpsimd.dma_start(q_all[:, _st, :], d_emb[0:P, :])
                  if STAGE != "ABW":
                    for st in range(NST):
                        nc.gpsimd.dma_gather(
                            out_ap=q_all[:, st:st + 1, :], in_ap=d_emb[:],
                            idxs_ap=idxs16[:, st * 8:(st + 1) * 8],
                            num_idxs=P, num_idxs_reg=P, elem_size=EMB_DIM)

                for st in range(NST if STAGE != "AB" else 0):
                    delta = fp.tile([P, EMB_DIM], f32, tag="delta")
                    nc.vector.tensor_tensor(out=delta[:], in0=q_all[:, st, :],
                                            in1=pooled_nat[:, st, :],
                                            op=Alu.subtract)
                    dscr = fp.tile([P, EMB_DIM], f32, tag="dscr")
                    nc.vector._custom_dve(
                        TENSOR_TENSOR_REDUCE, out=dscr[:], in0=delta[:],
                        in1=delta[:], s0=0.0, s1=1.0,
                        accum_out=sq_acc[:, st:st + 1])
                    qst = fp.tile([P, EMB_DIM], f32, tag="qst")
                    nc.vector.tensor_tensor(out=qst[:], in0=pooled_nat[:, st, :],
                                            in1=delta[:], op=Alu.add)
                    for r in range(SPAN):
                        nc.sync.dma_start(
                            d_qout[:].rearrange("(s e) d -> s e d", e=SPAN)
                            [st * P:(st + 1) * P, r, :],
                            qst[:])

                # boundaries: (logit + bb) > 0, transposed to token order
                bnd_all = fp.tile([P, NTT], f32, tag="bnd")
                nc.vector.tensor_scalar(out=bnd_all[:], in0=logit_all[:],
                                        scalar1=bb_sb[:, 0:1], scalar2=0.0,
                                        op0=Alu.add, op1=Alu.is_gt)
                ps_b = tp.tile([P, P], f32, tag="tp")
                nc.tensor.transpose(out=ps_b[:], in_=bnd_all[:], identity=ident[:])
                bndT = fp.tile([P, NTT], f32, tag="bndT")
                nc.scalar.mul(bndT[:], ps_b[:], 1.0)
                nc.sync.dma_start(d_bnd[:], bndT[:])
                nc.sync.dma_start(d_sq[:], sq_acc[:])

    nc.compile()
    return nc


def _get_nc():
    if "nc" not in _CACHE:
        _CACHE["nc"] = _build()
    return _CACHE["nc"]


def _prep_shared(emb, Wb, bb):
    e2T = np.ascontiguousarray((2.0 * emb).T).astype(np.float32)
    esq_row = np.sum(emb.astype(np.float32) ** 2, axis=1, dtype=np.float32)
    esq = np.ascontiguousarray(np.broadcast_to(esq_row, (P, NUM_EMB))).astype(np.float32)
    wb = np.ascontiguousarray(np.broadcast_to(Wb[:, 0], (P, EMB_DIM))).astype(np.float32)
    bbv = np.full((P, 1), np.float32(bb[0]), dtype=np.float32)
    p8 = np.zeros((P, SEGS_PER_TT), dtype=np.float32)
    for t in range(P):
        p8[t, t // SPAN] = 1.0 / SPAN
    return e2T, esq, wb, bbv, p8


def _run_cores(x, emb, Wb, bb):
    from concourse.bass_utils import run_bass_kernel_spmd
    nc = _get_nc()
    e2T, esq, wb, bbv, p8 = _prep_shared(
        np.asarray(emb, dtype=np.float32),
        np.asarray(Wb, dtype=np.float32),
        np.asarray(bb, dtype=np.float32))
    embf = np.ascontiguousarray(np.asarray(emb, dtype=np.float32))
    in_maps = []
    for k in range(NCORES):
        xs = np.ascontiguousarray(
            np.asarray(x[k * B_SH:(k + 1) * B_SH], dtype=np.float32)
        ).reshape(TOK, EMB_DIM)
        in_maps.append({"x": xs, "e2T": e2T, "emb": embf, "esq": esq,
                        "wb": wb, "bb": bbv, "p8": p8})
    res = run_bass_kernel_spmd(nc, in_maps, core_ids=list(range(NCORES)))
    return res.results


def kernel(x, emb, Wb, bb):
    x = np.asarray(x, dtype=np.float32)
    results = _run_cores(x, emb, Wb, bb)

    qout = np.empty((B, T, EMB_DIM), dtype=np.float32)
    idx_out = np.empty((B, T), dtype=np.int32)
    bnd_out = np.empty((B, T), dtype=np.float32)
    sq_total = 0.0
    bnd_total = 0.0
    for k, r in enumerate(results):
        qout[k * B_SH:(k + 1) * B_SH] = r["qout"].reshape(B_SH, T, EMB_DIM)
        idx_out[k * B_SH:(k + 1) * B_SH] = r["idx"].reshape(B_SH, T)
        bnd = r["bnd"].reshape(TOK)
        bnd_out[k * B_SH:(k + 1) * B_SH] = bnd.reshape(B_SH, T)
        sq_total += float(r["sq"].astype(np.float64).sum())
        bnd_total += float(bnd.astype(np.float64).sum())

    e_latent = sq_total / (B * (T // SPAN) * EMB_DIM)
    vq_loss = COMMIT * e_latent
    brate = bnd_total / (B * T)
    bloss = (brate - 1.0 / SPAN) ** 2
    total_loss = np.float32(vq_loss + BOUND_W * bloss)
    return qout, total_loss, idx_out, bnd_out
